# revision 9
# baseline (speedup 1.0000x reference)
"""Trainium2 Bass kernel for nn_Disout (block-dropout w/ global stats).

Strategy (8 NeuronCores, data-parallel over batch, 2 images/core):

Pass 1 (per core):
  - randdist is streamed in [w-partition, (h,c)-free] halo tiles.
    Seed values s' = fl32(fl32(rd + K) - 1) are computed with one dual-op
    tensor_scalar (K per-partition: interior fl(1-sdr), border fl(2-sdr);
    h-border columns overwritten with the border K) -> bf16. Sign of s'
    is bit-exact vs the reference's  fl32(K + rd) >= 1  test (Sterbenz).
  - 6-wide min-pool along h via log-trick (3 shifted tensor_tensor mins),
    fused clamp min(.,0) in the last step: U = min(hmin, 0) <= 0, and
    U == 0 iff all six seeds keep.
  - 6-wide min-pool along w == "sum of U over window is 0", done on the
    idle TensorEngine as a banded matmul (band weights 1.0, bf16).
  - block_pattern = (psum >= 0) -> uint8, with fused per-partition count
    accumulation (accum_out) for percent_ones. bp goes to a DRAM scratch.
  - x is streamed in flat tiles; ScalarE Square activation with fused
    accum_out gives sum(x^2) (mean^2 ~ 1e-7 of var -> dropped).
  - Tiny AllReduce (2 floats) across the 8 cores.

Pass 2 (per core): out = where(bp, x * (1/p), noise * (0.01*sqrt(var)/p)),
  via two ScalarE scaled copies + one VectorE copy_predicated, streamed.
"""

import os
import numpy as np
import ml_dtypes
from contextlib import ExitStack

import concourse.bacc as bacc
import concourse.bass as bass
import concourse.tile as tile
from concourse import mybir, bass_isa
from concourse.bass_utils import run_bass_kernel_spmd

AF = mybir.ActivationFunctionType
ALU = mybir.AluOpType
F32 = mybir.dt.float32
BF16 = mybir.dt.bfloat16
U8 = mybir.dt.uint8

B, W, H, C = 16, 224, 224, 64
NCORES = 8
BL = B // NCORES  # images per core
BS = 6
SDR = 0.1 * float(W * H) / (BS**2) / float((W - BS + 1) * (H - BS + 1))
K_INT = np.float32(1.0 - SDR)
K_BOR = np.float32(np.float32(1.0) + K_INT)
NF = float(B * W * H * C)

# (in0, in1, out0, out1) in global w/h coords; halo of 2 lo / 3 hi
W_TILES = [(0, 115, 0, 112), (110, 224, 112, 224)]
H_CHUNKS = [(0, 115, 0, 112), (110, 224, 112, 224)]
SMAX = 115 * C  # 7360
UMAX = 112 * C  # 7168
NMM = UMAX // 512  # 14 matmul n-chunks per unit
XF = 784  # flat free chunk (pass1 x-stats and pass2)
NXCH = (BL * W * H * C) // (128 * XF)  # 64

_NC = None


def _band(p, lo, hi):
    km = np.arange(p)[:, None] - np.arange(112)[None, :]
    return ((km >= lo) & (km <= hi)).astype(ml_dtypes.bfloat16)


def _kvec(w0, w1):
    wg = np.arange(w0, w1)
    return np.where((wg < 3) | (wg >= W - 2), K_BOR, K_INT).astype(np.float32)[:, None]


def _emit(nc, tc, ctx, X, RD, NS, OUT, DBG=None):
    x2 = X.rearrange("b w h c -> (b w h c)").rearrange("(p f) -> p f", p=128)
    ns2 = NS.rearrange("b w h c -> (b w h c)").rearrange("(p f) -> p f", p=128)
    out2 = OUT.rearrange("b w h c -> (b w h c)").rearrange("(p f) -> p f", p=128)

    consts = ctx.enter_context(tc.tile_pool(name="consts", bufs=1))
    p_rd = ctx.enter_context(tc.tile_pool(name="rd", bufs=2))
    p_bf = ctx.enter_context(tc.tile_pool(name="bf16", bufs=2))
    p_ps = ctx.enter_context(tc.tile_pool(name="ps", bufs=4, space="PSUM"))
    p_bp = ctx.enter_context(tc.tile_pool(name="bp", bufs=2))
    p_x1 = ctx.enter_context(tc.tile_pool(name="x1", bufs=3))
    p_dram = ctx.enter_context(tc.tile_pool(name="scratch", bufs=1, space="DRAM"))
    p_p2 = ctx.enter_context(tc.tile_pool(name="p2", bufs=3))

    # constants
    bands = {}
    kvecs = {}
    for ti, (w0, w1, _, _) in enumerate(W_TILES):
        p = w1 - w0
        lo, hi = (-2, 3) if ti == 0 else (0, 5)
        bt = consts.tile([p, 112], BF16, tag=f"band{ti}")
        band_c = nc.inline_tensor(_band(p, lo, hi), name=f"band_c{ti}")
        nc.sync.dma_start(out=bt, in_=band_c.ap())
        bands[ti] = bt
        kt = consts.tile([p, 1], F32, tag=f"kvec{ti}")
        kvec_c = nc.inline_tensor(_kvec(w0, w1), name=f"kvec_c{ti}")
        nc.sync.dma_start(out=kt, in_=kvec_c.ap())
        kvecs[ti] = kt

    cnts = consts.tile([112, 8 * NMM], F32, tag="cnts")
    xsqc = consts.tile([128, NXCH], F32, tag="xsqc")
    bp_dram = p_dram.tile([BL, W, H, C], U8)

    # ---------------- pass 1 ----------------
    unit = 0
    xch = 0

    def emit_xstat_chunk(j):
        xt = p_x1.tile([128, XF], F32, tag="xt")
        nc.sync.dma_start(out=xt, in_=x2[:, j * XF : (j + 1) * XF])
        sq = p_x1.tile([128, XF], BF16, tag="sq")
        nc.scalar.activation(
            out=sq, in_=xt, func=AF.Square, accum_out=xsqc[:, j : j + 1]
        )

    for b in range(BL):
        for ti, (win0, win1, wo0, wo1) in enumerate(W_TILES):
            P = win1 - win0
            for hi_, (a, bnd, o0, o1) in enumerate(H_CHUNKS):
                Hin = bnd - a
                Hout = o1 - o0

                rd_t = p_rd.tile([P, Hin * C], F32, tag="rd")
                nc.sync.dma_start(
                    out=rd_t,
                    in_=RD[b, win0:win1, a:bnd, :].rearrange("w h c -> w (h c)"),
                )
                # S = bf16((rd + K) - 1), K per-partition
                S = p_bf.tile([P, Hin * C], BF16, tag="s")
                nc.vector.tensor_scalar(
                    S, rd_t, kvecs[ti], 1.0, op0=ALU.add, op1=ALU.subtract
                )
                # h-border columns use K_BOR regardless of partition
                for hg in (0, 1, 2, W - 2, W - 1):
                    if a <= hg < bnd:
                        ls = (hg - a) * C
                        nc.vector.tensor_scalar(
                            S[:, ls : ls + C],
                            rd_t[:, ls : ls + C],
                            float(K_BOR),
                            1.0,
                            op0=ALU.add,
                            op1=ALU.subtract,
                        )
                # T1[h] = min(S[h], S[h+1])
                T1 = p_bf.tile([P, Hin * C], BF16, tag="t")
                nc.vector.tensor_tensor(
                    T1[:, : (Hin - 1) * C],
                    S[:, : (Hin - 1) * C],
                    S[:, C : Hin * C],
                    ALU.min,
                )
                T1n = Hin - 1
                if bnd == H:
                    nc.vector.tensor_copy(
                        T1[:, (Hin - 1) * C : Hin * C], S[:, (Hin - 1) * C : Hin * C]
                    )
                    T1n = Hin
                # T2[h] = min(T1[h], T1[h+2])
                T2 = p_bf.tile([P, Hin * C], BF16, tag="s")
                nc.vector.tensor_tensor(
                    T2[:, : (T1n - 2) * C],
                    T1[:, : (T1n - 2) * C],
                    T1[:, 2 * C : T1n * C],
                    ALU.min,
                )
                T2n = T1n - 2
                if bnd == H:
                    nc.vector.tensor_copy(
                        T2[:, (T1n - 2) * C : T1n * C], T1[:, (T1n - 2) * C : T1n * C]
                    )
                    T2n = T1n
                # U[j] = min(T2[j+d-2], 0, T2[j+d]), j = h-o0, d = o0-a
                U = p_bf.tile([P, Hout * C], BF16, tag="t")
                d = o0 - a
                g0 = max(o0, 2)
                j0 = g0 - o0
                i0 = g0 - a
                nm = o1 - g0
                nc.vector.scalar_tensor_tensor(
                    out=U[:, j0 * C : (j0 + nm) * C],
                    in0=T2[:, (i0 - 2) * C : (i0 - 2 + nm) * C],
                    scalar=0.0,
                    in1=T2[:, i0 * C : (i0 + nm) * C],
                    op0=ALU.min,
                    op1=ALU.min,
                )
                if o0 == 0:
                    nc.vector.tensor_scalar_min(U[:, 0:C], T2[:, 0:C], 0.0)
                    nc.vector.scalar_tensor_tensor(
                        out=U[:, C : 2 * C],
                        in0=T2[:, 0:C],
                        scalar=0.0,
                        in1=T2[:, C : 2 * C],
                        op0=ALU.min,
                        op1=ALU.min,
                    )
                # banded matmul over w + threshold + count
                bp_t = p_bp.tile([112, Hout * C], U8, tag="bp")
                for n in range(NMM):
                    ps = p_ps.tile([112, 512], F32, tag="ps")
                    nc.tensor.matmul(
                        ps,
                        lhsT=bands[ti],
                        rhs=U[:, n * 512 : (n + 1) * 512],
                        start=True,
                        stop=True,
                    )
                    nc.vector.tensor_scalar(
                        bp_t[:, n * 512 : (n + 1) * 512],
                        ps,
                        0.0,
                        None,
                        op0=ALU.is_ge,
                        op1=ALU.add,
                        accum_out=cnts[:, unit * NMM + n : unit * NMM + n + 1],
                    )
                nc.sync.dma_start(
                    out=bp_dram[b, wo0:wo1, o0:o1, :].rearrange("w h c -> w (h c)"),
                    in_=bp_t,
                )
                unit += 1
                for _ in range(NXCH // 8):
                    emit_xstat_chunk(xch)
                    xch += 1
    assert unit == 8 and xch == NXCH

    # ---------------- stats + allreduce ----------------
    cc_in = nc.dram_tensor("cc_in", [1, 2], F32, kind="Internal").ap()
    cc_out = nc.dram_tensor(
        "cc_out", [1, 2], F32, kind="Internal", addr_space="Shared"
    ).ap()

    sc = ctx.enter_context(tc.tile_pool(name="scalars", bufs=1))
    cnt_r = sc.tile([112, 1], F32, tag="cnt_r")
    nc.vector.tensor_reduce(cnt_r, cnts, axis=mybir.AxisListType.X, op=ALU.add)
    cnt_ar = sc.tile([112, 1], F32, tag="cnt_ar")
    nc.gpsimd.partition_all_reduce(cnt_ar, cnt_r, channels=112, reduce_op=bass_isa.ReduceOp.add)
    xsq_r = sc.tile([128, 1], F32, tag="xsq_r")
    nc.vector.tensor_reduce(xsq_r, xsqc, axis=mybir.AxisListType.X, op=ALU.add)
    xsq_ar = sc.tile([128, 1], F32, tag="xsq_ar")
    nc.gpsimd.partition_all_reduce(xsq_ar, xsq_r, channels=128, reduce_op=bass_isa.ReduceOp.add)

    stats_sb = sc.tile([1, 2], F32, tag="stats_sb")
    nc.vector.tensor_copy(stats_sb[:, 0:1], cnt_ar[0:1, :])
    nc.vector.tensor_copy(stats_sb[:, 1:2], xsq_ar[0:1, :])
    nc.gpsimd.dma_start(out=cc_in, in_=stats_sb)
    nc.gpsimd.collective_compute(
        "AllReduce",
        ALU.add,
        ins=[cc_in],
        outs=[cc_out],
        replica_groups=[list(range(NCORES))],
    )
    tot = sc.tile([1, 2], F32, tag="tot")
    nc.gpsimd.dma_start(out=tot, in_=cc_out)

    r = sc.tile([1, 1], F32, tag="r")
    nc.vector.reciprocal(r, tot[:, 0:1])  # 1 / total_count
    inv_p = sc.tile([1, 1], F32, tag="inv_p")
    nc.vector.tensor_scalar_mul(inv_p, r, NF)  # 1/percent_ones
    var = sc.tile([1, 1], F32, tag="var")
    nc.vector.tensor_scalar_mul(var, tot[:, 1:2], 1.0 / NF)
    sqv = sc.tile([1, 1], F32, tag="sqv")
    nc.scalar.sqrt(sqv, var)
    tmp = sc.tile([1, 1], F32, tag="tmp")
    nc.vector.tensor_tensor(tmp, sqv, inv_p, ALU.mult)
    scale2 = sc.tile([1, 1], F32, tag="scale2")
    nc.vector.tensor_scalar_mul(scale2, tmp, 0.01)  # 0.01*sqrt(var)/p
    inv_p_b = sc.tile([128, 1], F32, tag="inv_p_b")
    nc.gpsimd.partition_broadcast(inv_p_b, inv_p)
    scale2_b = sc.tile([128, 1], F32, tag="scale2_b")
    nc.gpsimd.partition_broadcast(scale2_b, scale2)

    if DBG is not None:
        dbg_t = sc.tile([1, 8], F32, tag="dbg_t")
        nc.vector.tensor_copy(dbg_t[:, 0:1], cnt_ar[0:1, :])
        nc.vector.tensor_copy(dbg_t[:, 1:2], xsq_ar[0:1, :])
        nc.vector.tensor_copy(dbg_t[:, 2:3], tot[:, 0:1])
        nc.vector.tensor_copy(dbg_t[:, 3:4], tot[:, 1:2])
        nc.vector.tensor_copy(dbg_t[:, 4:5], inv_p)
        nc.vector.tensor_copy(dbg_t[:, 5:6], scale2)
        nc.vector.tensor_copy(dbg_t[:, 6:7], inv_p_b[96:97, :])
        nc.vector.tensor_copy(dbg_t[:, 7:8], scale2_b[96:97, :])
        nc.sync.dma_start(out=DBG, in_=dbg_t)

    # ---------------- pass 2 ----------------
    bp2d = bp_dram.rearrange("b w h c -> (b w h c)").rearrange("(p f) -> p f", p=128)
    for j in range(NXCH):
        cs = slice(j * XF, (j + 1) * XF)
        xt = p_p2.tile([128, XF], F32, tag="x")
        nc.sync.dma_start(out=xt, in_=x2[:, cs])
        nt = p_p2.tile([128, XF], F32, tag="n")
        nc.sync.dma_start(out=nt, in_=ns2[:, cs])
        bt = p_p2.tile([128, XF], U8, tag="b")
        nc.sync.dma_start(out=bt, in_=bp2d[:, cs])
        ot = p_p2.tile([128, XF], F32, tag="o")
        nc.scalar.activation(out=ot, in_=nt, func=AF.Copy, bias=0.0, scale=scale2_b)
        xs = p_p2.tile([128, XF], F32, tag="xs")
        nc.scalar.activation(out=xs, in_=xt, func=AF.Copy, bias=0.0, scale=inv_p_b)
        nc.vector.copy_predicated(out=ot, mask=bt, data=xs)
        nc.sync.dma_start(out=out2[:, cs], in_=ot)


def _build():
    nc = bacc.Bacc(
        "TRN2",
        target_bir_lowering=False,
        debug=False,
        enable_asserts=False,
        num_devices=NCORES,
    )
    X = nc.dram_tensor("x", [BL, W, H, C], F32, kind="ExternalInput").ap()
    RD = nc.dram_tensor("randdist", [BL, W, H, C], F32, kind="ExternalInput").ap()
    NS = nc.dram_tensor("noise", [BL, W, H, C], F32, kind="ExternalInput").ap()
    OUT = nc.dram_tensor("out", [BL, W, H, C], F32, kind="ExternalOutput").ap()
    DBG = None
    if int(os.environ.get("DISOUT_DEBUG", "0")):
        DBG = nc.dram_tensor("dbg", [1, 8], F32, kind="ExternalOutput").ap()
    with tile.TileContext(nc) as tc, ExitStack() as ctx:
        _emit(nc, tc, ctx, X, RD, NS, OUT, DBG)
    nc.compile()
    return nc


def kernel(x, randdist, noise):
    global _NC
    if _NC is None:
        _NC = _build()
    x = np.ascontiguousarray(x, dtype=np.float32)
    randdist = np.ascontiguousarray(randdist, dtype=np.float32)
    noise = np.ascontiguousarray(noise, dtype=np.float32)
    in_maps = [
        {
            "x": x[i * BL : (i + 1) * BL],
            "randdist": randdist[i * BL : (i + 1) * BL],
            "noise": noise[i * BL : (i + 1) * BL],
        }
        for i in range(NCORES)
    ]
    trace = bool(int(os.environ.get("DISOUT_TRACE", "0")))
    res = run_bass_kernel_spmd(
        _NC, in_maps, core_ids=list(range(NCORES)), trace=trace
    )
    if trace and res.exec_time_ns is not None:
        print(f"HW exec time: {res.exec_time_ns} ns")
        if res.instructions_and_trace is not None:
            print(f"trace: {res.instructions_and_trace[1]}")
    return np.concatenate([res.results[i]["out"] for i in range(NCORES)], axis=0)


# revision 35
# speedup vs baseline: 6.4069x; 6.4069x over previous
"""Trainium2 Bass kernel for nn_Disout (block-dropout w/ global stats).

Strategy (8 NeuronCores, data-parallel over batch, 2 images/core):

Pass 1 (per core):
  - randdist is streamed in [w-partition, (h,c)-free] halo tiles.
    Seed values s' = fl32(fl32(rd + K) - 1) are computed with one dual-op
    tensor_scalar (K per-partition: interior fl(1-sdr), border fl(2-sdr);
    h-border columns overwritten with the border K) -> bf16. Sign of s'
    is bit-exact vs the reference's  fl32(K + rd) >= 1  test (Sterbenz).
  - 6-wide min-pool along h via log-trick (3 shifted tensor_tensor mins),
    fused clamp min(.,0) in the last step: U = min(hmin, 0) <= 0, and
    U == 0 iff all six seeds keep.
  - 6-wide min-pool along w == "sum of U over window is 0", done on the
    idle TensorEngine as a banded matmul (band weights 1.0, bf16).
  - drop mask = Sign(psum) on ScalarE -> int8 {0:keep, -1:dropped}, with
    fused per-partition accumulation (accum_out) giving -(#dropped) for
    percent_ones. Mask goes to a DRAM scratch.
  - x is streamed in flat tiles; ScalarE Square activation with fused
    accum_out gives sum(x^2) (mean^2 ~ 1e-7 of var -> dropped).
  - Tiny AllReduce (2 floats) across the 8 cores.

Pass 2 (per core): base = x * (1/p) on ScalarE, noise-branch
  noise * (0.01*sqrt(var)/p) on ScalarE, VectorE copy_predicated
  overwrites dropped positions (mask nonzero), streamed flat.

Engine budget per core (est): DMA ~142 MB ≈ 394 us (the roofline);
DVE ~190 us; ACT ~170 us; PE ~50 us. Measured ~520 us (TimelineSim 467).
"""

import os
import numpy as np
import ml_dtypes
from contextlib import ExitStack

import concourse.bacc as bacc
import concourse.bass as bass
import concourse.tile as tile
from concourse import mybir, bass_isa
from concourse.bass_utils import run_bass_kernel_spmd

AF = mybir.ActivationFunctionType
ALU = mybir.AluOpType
F32 = mybir.dt.float32
BF16 = mybir.dt.bfloat16
U8 = mybir.dt.uint8
I8 = mybir.dt.int8

B, W, H, C = 16, 224, 224, 64
NCORES = 8
BL = B // NCORES  # images per core
BS = 6
SDR = 0.1 * float(W * H) / (BS**2) / float((W - BS + 1) * (H - BS + 1))
K_INT = np.float32(1.0 - SDR)
K_BOR = np.float32(np.float32(1.0) + K_INT)
NF = float(B * W * H * C)

# (in0, in1, out0, out1) in global w/h coords; halo of 2 lo / 3 hi
W_TILES = [(0, 115, 0, 112), (110, 224, 112, 224)]
H_CHUNKS = [(0, 115, 0, 112), (110, 224, 112, 224)]
SMAX = 115 * C  # 7360
UMAX = 112 * C  # 7168
NMM = UMAX // 512  # 14 matmul n-chunks per unit
XF = 784  # flat free chunk (pass1 x-stats and pass2)
NXCH = (BL * W * H * C) // (128 * XF)  # 64

_NC = None


def _band(p, lo, hi):
    km = np.arange(p)[:, None] - np.arange(112)[None, :]
    return ((km >= lo) & (km <= hi)).astype(ml_dtypes.bfloat16)


def _kvec(w0, w1):
    wg = np.arange(w0, w1)
    return np.where((wg < 3) | (wg >= W - 2), K_BOR, K_INT).astype(np.float32)[:, None]


def _emit(nc, tc, ctx, X, RD, NS, OUT, DBG=None, it=0):
    x2 = X.rearrange("b w h c -> (b w h c)").rearrange("(p f) -> p f", p=128)
    ns2 = NS.rearrange("b w h c -> (b w h c)").rearrange("(p f) -> p f", p=128)
    out2 = OUT.rearrange("b w h c -> (b w h c)").rearrange("(p f) -> p f", p=128)

    consts = ctx.enter_context(tc.tile_pool(name="consts", bufs=1))
    p1ctx = ExitStack()
    p_rd = p1ctx.enter_context(tc.tile_pool(name="rd", bufs=2))
    p_bf = p1ctx.enter_context(tc.tile_pool(name="bf16", bufs=2))
    p_ps = p1ctx.enter_context(tc.tile_pool(name="ps", bufs=4, space="PSUM"))
    p_x1 = p1ctx.enter_context(tc.tile_pool(name="x1", bufs=3))

    # constants
    bands = {}
    kvecs = {}
    for ti, (w0, w1, _, _) in enumerate(W_TILES):
        p = w1 - w0
        lo, hi = (-2, 3) if ti == 0 else (0, 5)
        bt = consts.tile([p, 112], BF16, tag=f"band{ti}")
        band_c = nc.inline_tensor(_band(p, lo, hi), name=f"band_c{ti}_{it}")
        nc.sync.dma_start(out=bt, in_=band_c.ap())
        bands[ti] = bt
        kt = consts.tile([p, 1], F32, tag=f"kvec{ti}")
        kvec_c = nc.inline_tensor(_kvec(w0, w1), name=f"kvec_c{ti}_{it}")
        nc.sync.dma_start(out=kt, in_=kvec_c.ap())
        kvecs[ti] = kt

    cnts = consts.tile([112, 8 * NMM], F32, tag="cnts")
    xsqc = consts.tile([128, NXCH], F32, tag="xsqc")
    # persistent drop mask, SBUF-resident across both passes (never in HBM)
    bp_all = consts.tile([112, 8 * UMAX], I8, tag="bp_all")

    # ---------------- pass 1 ----------------
    abl = os.environ.get("DISOUT_ABL", "")
    unit = 0
    xch = 0

    def emit_xstat_chunk(j):
        xt = p_x1.tile([128, XF], F32, tag="xt")
        nc.sync.dma_start(out=xt, in_=x2[:, j * XF : (j + 1) * XF])
        sq = p_x1.tile([128, XF], BF16, tag="sq")
        nc.scalar.activation(
            out=sq, in_=xt, func=AF.Square, accum_out=xsqc[:, j : j + 1]
        )

    if "nop1" in abl:
        nc.vector.memset(cnts, 1.0)
    if "nox" in abl:
        nc.vector.memset(xsqc, 1.0)
    for b in range(BL if "nop1" not in abl else 0):
        for ti, (win0, win1, wo0, wo1) in enumerate(W_TILES):
            P = win1 - win0
            for hi_, (a, bnd, o0, o1) in enumerate(H_CHUNKS):
                Hin = bnd - a
                Hout = o1 - o0

                rd_t = p_rd.tile([P, Hin * C], F32, tag="rd")
                nc.sync.dma_start(
                    out=rd_t,
                    in_=RD[b, win0:win1, a:bnd, :].rearrange("w h c -> w (h c)"),
                )
                # S = bf16((rd + K) - 1), K per-partition
                S = p_bf.tile([P, Hin * C], BF16, tag="s")
                nc.vector.tensor_scalar(
                    S, rd_t, kvecs[ti], 1.0, op0=ALU.add, op1=ALU.subtract
                )
                # h-border columns use K_BOR regardless of partition
                for hg in (0, 1, 2, W - 2, W - 1):
                    if a <= hg < bnd:
                        ls = (hg - a) * C
                        nc.vector.tensor_scalar(
                            S[:, ls : ls + C],
                            rd_t[:, ls : ls + C],
                            float(K_BOR),
                            1.0,
                            op0=ALU.add,
                            op1=ALU.subtract,
                        )
                # T1[h] = min(S[h], S[h+1])
                T1 = p_bf.tile([P, Hin * C], BF16, tag="t")
                nc.vector.tensor_tensor(
                    T1[:, : (Hin - 1) * C],
                    S[:, : (Hin - 1) * C],
                    S[:, C : Hin * C],
                    ALU.min,
                )
                T1n = Hin - 1
                if bnd == H:
                    nc.vector.tensor_copy(
                        T1[:, (Hin - 1) * C : Hin * C], S[:, (Hin - 1) * C : Hin * C]
                    )
                    T1n = Hin
                # T2[h] = min(T1[h], T1[h+2])
                T2 = p_bf.tile([P, Hin * C], BF16, tag="s")
                nc.vector.tensor_tensor(
                    T2[:, : (T1n - 2) * C],
                    T1[:, : (T1n - 2) * C],
                    T1[:, 2 * C : T1n * C],
                    ALU.min,
                )
                T2n = T1n - 2
                if bnd == H:
                    nc.vector.tensor_copy(
                        T2[:, (T1n - 2) * C : T1n * C], T1[:, (T1n - 2) * C : T1n * C]
                    )
                    T2n = T1n
                # U[j] = min(T2[j+d-2], 0, T2[j+d]), j = h-o0, d = o0-a
                U = p_bf.tile([P, Hout * C], BF16, tag="t")
                d = o0 - a
                g0 = max(o0, 2)
                j0 = g0 - o0
                i0 = g0 - a
                nm = o1 - g0
                nc.vector.scalar_tensor_tensor(
                    out=U[:, j0 * C : (j0 + nm) * C],
                    in0=T2[:, (i0 - 2) * C : (i0 - 2 + nm) * C],
                    scalar=0.0,
                    in1=T2[:, i0 * C : (i0 + nm) * C],
                    op0=ALU.min,
                    op1=ALU.min,
                )
                if o0 == 0:
                    nc.vector.tensor_scalar_min(U[:, 0:C], T2[:, 0:C], 0.0)
                    nc.vector.scalar_tensor_tensor(
                        out=U[:, C : 2 * C],
                        in0=T2[:, 0:C],
                        scalar=0.0,
                        in1=T2[:, C : 2 * C],
                        op0=ALU.min,
                        op1=ALU.min,
                    )
                # banded matmul over w + threshold + count
                # drop mask: Sign(psum) = 0 (keep, bp=1) / -1 (dropped, bp=0);
                # accum gives -(#dropped) per partition per chunk
                ub = unit * UMAX
                for n in range(NMM):
                    ps = p_ps.tile([112, 512], F32, tag="ps")
                    nc.tensor.matmul(
                        ps,
                        lhsT=bands[ti],
                        rhs=U[:, n * 512 : (n + 1) * 512],
                        start=True,
                        stop=True,
                    )
                    nc.scalar.activation(
                        bp_all[:, ub + n * 512 : ub + (n + 1) * 512],
                        ps,
                        AF.Sign,
                        accum_out=cnts[:, unit * NMM + n : unit * NMM + n + 1],
                    )
                unit += 1
                if "nox" not in abl:
                    for _ in range(NXCH // 8):
                        emit_xstat_chunk(xch)
                        xch += 1
    if "nox" not in abl:
        while xch < NXCH:
            emit_xstat_chunk(xch)
            xch += 1
    p1ctx.close()  # release pass-1 streaming pools; pass-2 pools reuse the SBUF

    # ---------------- stats + allreduce ----------------
    cc_in = nc.dram_tensor(f"cc_in{it}", [1, 2], F32, kind="Internal").ap()
    cc_out = nc.dram_tensor(
        f"cc_out{it}", [1, 2], F32, kind="Internal", addr_space="Shared"
    ).ap()

    sc = ctx.enter_context(tc.tile_pool(name="scalars", bufs=1))
    cnt_r = sc.tile([112, 1], F32, tag="cnt_r")
    nc.vector.tensor_reduce(cnt_r, cnts, axis=mybir.AxisListType.X, op=ALU.add)
    # cnts holds -(#dropped); keep count = elems/partition + sum
    nc.vector.tensor_scalar_add(cnt_r, cnt_r, float(BL * W * H * C // 112))
    cnt_ar = sc.tile([112, 1], F32, tag="cnt_ar")
    nc.gpsimd.partition_all_reduce(cnt_ar, cnt_r, channels=112, reduce_op=bass_isa.ReduceOp.add)
    xsq_r = sc.tile([128, 1], F32, tag="xsq_r")
    nc.vector.tensor_reduce(xsq_r, xsqc, axis=mybir.AxisListType.X, op=ALU.add)
    xsq_ar = sc.tile([128, 1], F32, tag="xsq_ar")
    nc.gpsimd.partition_all_reduce(xsq_ar, xsq_r, channels=128, reduce_op=bass_isa.ReduceOp.add)

    stats_sb = sc.tile([1, 2], F32, tag="stats_sb")
    nc.vector.tensor_copy(stats_sb[:, 0:1], cnt_ar[0:1, :])
    nc.vector.tensor_copy(stats_sb[:, 1:2], xsq_ar[0:1, :])
    tot = sc.tile([1, 2], F32, tag="tot")
    if int(os.environ.get("DISOUT_NOCC", "0")):
        # single-core / cost-model builds: skip the collective
        nc.vector.tensor_copy(tot, stats_sb)
        nc.vector.tensor_scalar_mul(tot, tot, float(NCORES))
    else:
        nc.gpsimd.dma_start(out=cc_in, in_=stats_sb)
        nc.gpsimd.collective_compute(
            "AllReduce",
            ALU.add,
            ins=[cc_in],
            outs=[cc_out],
            replica_groups=[list(range(NCORES))],
        )
        nc.gpsimd.dma_start(out=tot, in_=cc_out)

    r = sc.tile([1, 1], F32, tag="r")
    nc.vector.reciprocal(r, tot[:, 0:1])  # 1 / total_count
    inv_p = sc.tile([1, 1], F32, tag="inv_p")
    nc.vector.tensor_scalar_mul(inv_p, r, NF)  # 1/percent_ones
    var = sc.tile([1, 1], F32, tag="var")
    nc.vector.tensor_scalar_mul(var, tot[:, 1:2], 1.0 / NF)
    sqv = sc.tile([1, 1], F32, tag="sqv")
    nc.scalar.sqrt(sqv, var)
    tmp = sc.tile([1, 1], F32, tag="tmp")
    nc.vector.tensor_tensor(tmp, sqv, inv_p, ALU.mult)
    scale2 = sc.tile([1, 1], F32, tag="scale2")
    nc.vector.tensor_scalar_mul(scale2, tmp, 0.01)  # 0.01*sqrt(var)/p
    inv_p_b = sc.tile([128, 1], F32, tag="inv_p_b")
    nc.gpsimd.partition_broadcast(inv_p_b, inv_p)
    scale2_b = sc.tile([128, 1], F32, tag="scale2_b")
    nc.gpsimd.partition_broadcast(scale2_b, scale2)

    if DBG is not None:
        dbg_t = sc.tile([1, 8], F32, tag="dbg_t")
        nc.vector.tensor_copy(dbg_t[:, 0:1], cnt_ar[0:1, :])
        nc.vector.tensor_copy(dbg_t[:, 1:2], xsq_ar[0:1, :])
        nc.vector.tensor_copy(dbg_t[:, 2:3], tot[:, 0:1])
        nc.vector.tensor_copy(dbg_t[:, 3:4], tot[:, 1:2])
        nc.vector.tensor_copy(dbg_t[:, 4:5], inv_p)
        nc.vector.tensor_copy(dbg_t[:, 5:6], scale2)
        nc.vector.tensor_copy(dbg_t[:, 6:7], inv_p_b[96:97, :])
        nc.vector.tensor_copy(dbg_t[:, 7:8], scale2_b[96:97, :])
        nc.sync.dma_start(out=DBG, in_=dbg_t)

    # ---------------- pass 2 (unit layout; mask stays in SBUF) ----------------
    p_p2 = ctx.enter_context(tc.tile_pool(name="p2", bufs=2))
    HF = Hout2 = (UMAX // 2)  # 3584 elems = half an (h,c) chunk
    if "nop2" not in abl:
        unit = 0
        for b in range(BL):
            for ti, (win0, win1, wo0, wo1) in enumerate(W_TILES):
                for hi_, (a, bnd, o0, o1) in enumerate(H_CHUNKS):
                    base = o0 * C
                    ub = unit * UMAX
                    for half in range(2):
                        cs = slice(base + half * HF, base + (half + 1) * HF)
                        xt = p_p2.tile([112, HF], F32, tag="x")
                        nc.sync.dma_start(
                            out=xt,
                            in_=X[b, wo0:wo1].rearrange("w h c -> w (h c)")[:, cs],
                        )
                        nt = p_p2.tile([112, HF], F32, tag="n")
                        nc.sync.dma_start(
                            out=nt,
                            in_=NS[b, wo0:wo1].rearrange("w h c -> w (h c)")[:, cs],
                        )
                        # base = keep-branch x/p; dropped (mask=-1) <- noise branch
                        ot = p_p2.tile([112, HF], F32, tag="o")
                        nc.scalar.activation(
                            out=ot, in_=xt, func=AF.Copy, bias=0.0,
                            scale=inv_p_b[0:112],
                        )
                        cn = p_p2.tile([112, HF], F32, tag="cn")
                        nc.scalar.activation(
                            out=cn, in_=nt, func=AF.Copy, bias=0.0,
                            scale=scale2_b[0:112],
                        )
                        nc.vector.copy_predicated(
                            out=ot,
                            mask=bp_all[:, ub + half * HF : ub + (half + 1) * HF],
                            data=cn,
                        )
                        nc.sync.dma_start(
                            out=OUT[b, wo0:wo1].rearrange("w h c -> w (h c)")[:, cs],
                            in_=ot,
                        )
                    unit += 1


def _build(iters=1):
    nc = bacc.Bacc(
        "TRN2",
        target_bir_lowering=False,
        debug=False,
        enable_asserts=False,
        num_devices=NCORES,
    )
    X = nc.dram_tensor("x", [BL, W, H, C], F32, kind="ExternalInput").ap()
    RD = nc.dram_tensor("randdist", [BL, W, H, C], F32, kind="ExternalInput").ap()
    NS = nc.dram_tensor("noise", [BL, W, H, C], F32, kind="ExternalInput").ap()
    OUT = nc.dram_tensor("out", [BL, W, H, C], F32, kind="ExternalOutput").ap()
    DBG = None
    if int(os.environ.get("DISOUT_DEBUG", "0")):
        DBG = nc.dram_tensor("dbg", [1, 8], F32, kind="ExternalOutput").ap()
    with tile.TileContext(nc) as tc:
        for it in range(iters):
            with ExitStack() as ctx:
                _emit(nc, tc, ctx, X, RD, NS, OUT, DBG, it=it)
    nc.compile()
    return nc


def kernel(x, randdist, noise):
    global _NC
    if _NC is None:
        _NC = _build()
    x = np.ascontiguousarray(x, dtype=np.float32)
    randdist = np.ascontiguousarray(randdist, dtype=np.float32)
    noise = np.ascontiguousarray(noise, dtype=np.float32)
    in_maps = [
        {
            "x": x[i * BL : (i + 1) * BL],
            "randdist": randdist[i * BL : (i + 1) * BL],
            "noise": noise[i * BL : (i + 1) * BL],
        }
        for i in range(NCORES)
    ]
    trace = bool(int(os.environ.get("DISOUT_TRACE", "0")))
    res = run_bass_kernel_spmd(
        _NC, in_maps, core_ids=list(range(NCORES)), trace=trace
    )
    if trace and res.exec_time_ns is not None:
        print(f"HW exec time: {res.exec_time_ns} ns")
        if res.instructions_and_trace is not None:
            print(f"trace: {res.instructions_and_trace[1]}")
    return np.concatenate([res.results[i]["out"] for i in range(NCORES)], axis=0)


# revision 41
# speedup vs baseline: 13.1151x; 2.0470x over previous
"""Trainium2 Bass kernel for nn_Disout (block-dropout w/ global stats).

Strategy (8 NeuronCores, data-parallel over batch, 2 images/core):

Pass 1 (per core):
  - randdist is streamed in [w-partition, (h,c)-free] halo tiles.
    Seed values s' = fl32(fl32(rd + K) - 1) are computed with one dual-op
    tensor_scalar (K per-partition: interior fl(1-sdr), border fl(2-sdr);
    h-border columns overwritten with the border K) -> bf16. Sign of s'
    is bit-exact vs the reference's  fl32(K + rd) >= 1  test (Sterbenz).
  - 6-wide min-pool along h via log-trick (3 shifted tensor_tensor mins),
    fused clamp min(.,0) in the last step: U = min(hmin, 0) <= 0, and
    U == 0 iff all six seeds keep.
  - 6-wide min-pool along w == "sum of U over window is 0", done on the
    idle TensorEngine as a banded matmul (band weights 1.0, bf16).
  - drop mask = Sign(psum) on ScalarE -> int8 {0:keep, -1:dropped}, with
    fused per-partition accumulation (accum_out) giving -(#dropped) for
    percent_ones. Mask goes to a DRAM scratch.
  - x is streamed in flat tiles; ScalarE Square activation with fused
    accum_out gives sum(x^2) (mean^2 ~ 1e-7 of var -> dropped).
  - Tiny AllReduce (2 floats) across the 8 cores.

Pass 2 (per core): base = x * (1/p) on ScalarE, noise-branch
  noise * (0.01*sqrt(var)/p) on ScalarE, VectorE copy_predicated
  overwrites dropped positions (mask nonzero), streamed flat.

Engine budget per core (est): DMA ~142 MB ≈ 394 us (the roofline);
DVE ~190 us; ACT ~170 us; PE ~50 us. Measured 453-459 us on HW
(in-NEFF repetition differencing); TimelineSim predicts 467 us.
Note: an SBUF-resident-mask variant (no bp DRAM round trip, sim 425)
measured 1.66x SLOWER on real HW - the pass-1->pass-2 pool-release
barrier outweighs the 12.8 MB saved; kept the DRAM-scratch version.
"""

import os
import numpy as np
import ml_dtypes
from contextlib import ExitStack

import concourse.bacc as bacc
import concourse.bass as bass
import concourse.tile as tile
from concourse import mybir, bass_isa
from concourse.bass_utils import run_bass_kernel_spmd

AF = mybir.ActivationFunctionType
ALU = mybir.AluOpType
F32 = mybir.dt.float32
BF16 = mybir.dt.bfloat16
U8 = mybir.dt.uint8
I8 = mybir.dt.int8

B, W, H, C = 16, 224, 224, 64
NCORES = 8
BL = B // NCORES  # images per core
BS = 6
SDR = 0.1 * float(W * H) / (BS**2) / float((W - BS + 1) * (H - BS + 1))
K_INT = np.float32(1.0 - SDR)
K_BOR = np.float32(np.float32(1.0) + K_INT)
NF = float(B * W * H * C)

# (in0, in1, out0, out1) in global w/h coords; halo of 2 lo / 3 hi
W_TILES = [(0, 115, 0, 112), (110, 224, 112, 224)]
H_CHUNKS = [(0, 115, 0, 112), (110, 224, 112, 224)]
SMAX = 115 * C  # 7360
UMAX = 112 * C  # 7168
NMM = UMAX // 512  # 14 matmul n-chunks per unit
XF = 784  # flat free chunk (pass1 x-stats and pass2)
NXCH = (BL * W * H * C) // (128 * XF)  # 64

_NC = None


def _band(p, lo, hi):
    km = np.arange(p)[:, None] - np.arange(112)[None, :]
    return ((km >= lo) & (km <= hi)).astype(ml_dtypes.bfloat16)


def _kvec(w0, w1):
    wg = np.arange(w0, w1)
    return np.where((wg < 3) | (wg >= W - 2), K_BOR, K_INT).astype(np.float32)[:, None]


def _emit(nc, tc, ctx, X, RD, NS, OUT, DBG=None, it=0):
    x2 = X.rearrange("b w h c -> (b w h c)").rearrange("(p f) -> p f", p=128)
    ns2 = NS.rearrange("b w h c -> (b w h c)").rearrange("(p f) -> p f", p=128)
    out2 = OUT.rearrange("b w h c -> (b w h c)").rearrange("(p f) -> p f", p=128)

    consts = ctx.enter_context(tc.tile_pool(name="consts", bufs=1))
    p_rd = ctx.enter_context(tc.tile_pool(name="rd", bufs=2))
    p_bf = ctx.enter_context(tc.tile_pool(name="bf16", bufs=2))
    p_ps = ctx.enter_context(tc.tile_pool(name="ps", bufs=4, space="PSUM"))
    p_bp = ctx.enter_context(tc.tile_pool(name="bp", bufs=2))
    p_x1 = ctx.enter_context(tc.tile_pool(name="x1", bufs=2))
    p_dram = ctx.enter_context(tc.tile_pool(name="scratch", bufs=1, space="DRAM"))
    p_p2 = ctx.enter_context(tc.tile_pool(name="p2", bufs=2))

    # constants
    bands = {}
    kvecs = {}
    for ti, (w0, w1, _, _) in enumerate(W_TILES):
        p = w1 - w0
        lo, hi = (-2, 3) if ti == 0 else (0, 5)
        bt = consts.tile([p, 112], BF16, tag=f"band{ti}")
        band_c = nc.inline_tensor(_band(p, lo, hi), name=f"band_c{ti}_{it}")
        nc.sync.dma_start(out=bt, in_=band_c.ap())
        bands[ti] = bt
        kt = consts.tile([p, 1], F32, tag=f"kvec{ti}")
        kvec_c = nc.inline_tensor(_kvec(w0, w1), name=f"kvec_c{ti}_{it}")
        nc.sync.dma_start(out=kt, in_=kvec_c.ap())
        kvecs[ti] = kt

    cnts = consts.tile([112, 8 * NMM], F32, tag="cnts")
    xsqc = consts.tile([128, NXCH], F32, tag="xsqc")
    bp_dram = p_dram.tile([BL, W, H, C], I8)

    # ---------------- pass 1 ----------------
    abl = os.environ.get("DISOUT_ABL", "")
    unit = 0
    xch = 0

    def emit_xstat_chunk(j):
        xt = p_x1.tile([128, XF], F32, tag="xt")
        nc.sync.dma_start(out=xt, in_=x2[:, j * XF : (j + 1) * XF])
        sq = p_x1.tile([128, XF], BF16, tag="sq")
        nc.scalar.activation(
            out=sq, in_=xt, func=AF.Square, accum_out=xsqc[:, j : j + 1]
        )

    if "nop1" in abl:
        nc.vector.memset(cnts, 1.0)
    if "nox" in abl:
        nc.vector.memset(xsqc, 1.0)
    for b in range(BL if "nop1" not in abl else 0):
        for ti, (win0, win1, wo0, wo1) in enumerate(W_TILES):
            P = win1 - win0
            for hi_, (a, bnd, o0, o1) in enumerate(H_CHUNKS):
                Hin = bnd - a
                Hout = o1 - o0

                rd_t = p_rd.tile([P, Hin * C], F32, tag="rd")
                nc.sync.dma_start(
                    out=rd_t,
                    in_=RD[b, win0:win1, a:bnd, :].rearrange("w h c -> w (h c)"),
                )
                # S = bf16((rd + K) - 1), K per-partition
                S = p_bf.tile([P, Hin * C], BF16, tag="s")
                nc.vector.tensor_scalar(
                    S, rd_t, kvecs[ti], 1.0, op0=ALU.add, op1=ALU.subtract
                )
                # h-border columns use K_BOR regardless of partition
                for hg in (0, 1, 2, W - 2, W - 1):
                    if a <= hg < bnd:
                        ls = (hg - a) * C
                        nc.vector.tensor_scalar(
                            S[:, ls : ls + C],
                            rd_t[:, ls : ls + C],
                            float(K_BOR),
                            1.0,
                            op0=ALU.add,
                            op1=ALU.subtract,
                        )
                # T1[h] = min(S[h], S[h+1])
                T1 = p_bf.tile([P, Hin * C], BF16, tag="t")
                nc.vector.tensor_tensor(
                    T1[:, : (Hin - 1) * C],
                    S[:, : (Hin - 1) * C],
                    S[:, C : Hin * C],
                    ALU.min,
                )
                T1n = Hin - 1
                if bnd == H:
                    nc.vector.tensor_copy(
                        T1[:, (Hin - 1) * C : Hin * C], S[:, (Hin - 1) * C : Hin * C]
                    )
                    T1n = Hin
                # T2[h] = min(T1[h], T1[h+2])
                T2 = p_bf.tile([P, Hin * C], BF16, tag="s")
                nc.vector.tensor_tensor(
                    T2[:, : (T1n - 2) * C],
                    T1[:, : (T1n - 2) * C],
                    T1[:, 2 * C : T1n * C],
                    ALU.min,
                )
                T2n = T1n - 2
                if bnd == H:
                    nc.vector.tensor_copy(
                        T2[:, (T1n - 2) * C : T1n * C], T1[:, (T1n - 2) * C : T1n * C]
                    )
                    T2n = T1n
                # U[j] = min(T2[j+d-2], 0, T2[j+d]), j = h-o0, d = o0-a
                U = p_bf.tile([P, Hout * C], BF16, tag="t")
                d = o0 - a
                g0 = max(o0, 2)
                j0 = g0 - o0
                i0 = g0 - a
                nm = o1 - g0
                nc.vector.scalar_tensor_tensor(
                    out=U[:, j0 * C : (j0 + nm) * C],
                    in0=T2[:, (i0 - 2) * C : (i0 - 2 + nm) * C],
                    scalar=0.0,
                    in1=T2[:, i0 * C : (i0 + nm) * C],
                    op0=ALU.min,
                    op1=ALU.min,
                )
                if o0 == 0:
                    nc.vector.tensor_scalar_min(U[:, 0:C], T2[:, 0:C], 0.0)
                    nc.vector.scalar_tensor_tensor(
                        out=U[:, C : 2 * C],
                        in0=T2[:, 0:C],
                        scalar=0.0,
                        in1=T2[:, C : 2 * C],
                        op0=ALU.min,
                        op1=ALU.min,
                    )
                # banded matmul over w + threshold + count
                # drop mask: Sign(psum) = 0 (keep, bp=1) / -1 (dropped, bp=0);
                # accum gives -(#dropped) per partition per chunk
                bp_t = p_bp.tile([112, Hout * C], I8, tag="bp")
                for n in range(NMM):
                    ps = p_ps.tile([112, 512], F32, tag="ps")
                    nc.tensor.matmul(
                        ps,
                        lhsT=bands[ti],
                        rhs=U[:, n * 512 : (n + 1) * 512],
                        start=True,
                        stop=True,
                    )
                    nc.scalar.activation(
                        bp_t[:, n * 512 : (n + 1) * 512],
                        ps,
                        AF.Sign,
                        accum_out=cnts[:, unit * NMM + n : unit * NMM + n + 1],
                    )
                nc.sync.dma_start(
                    out=bp_dram[b, wo0:wo1, o0:o1, :].rearrange("w h c -> w (h c)"),
                    in_=bp_t,
                )
                unit += 1
                if "nox" not in abl:
                    for _ in range(NXCH // 8):
                        emit_xstat_chunk(xch)
                        xch += 1
    if "nox" not in abl:
        while xch < NXCH:
            emit_xstat_chunk(xch)
            xch += 1

    # ---------------- stats + allreduce ----------------
    cc_in = nc.dram_tensor(f"cc_in{it}", [1, 2], F32, kind="Internal").ap()
    cc_out = nc.dram_tensor(
        f"cc_out{it}", [1, 2], F32, kind="Internal", addr_space="Shared"
    ).ap()

    sc = ctx.enter_context(tc.tile_pool(name="scalars", bufs=1))
    cnt_r = sc.tile([112, 1], F32, tag="cnt_r")
    nc.vector.tensor_reduce(cnt_r, cnts, axis=mybir.AxisListType.X, op=ALU.add)
    # cnts holds -(#dropped); keep count = elems/partition + sum
    nc.vector.tensor_scalar_add(cnt_r, cnt_r, float(BL * W * H * C // 112))
    cnt_ar = sc.tile([112, 1], F32, tag="cnt_ar")
    nc.gpsimd.partition_all_reduce(cnt_ar, cnt_r, channels=112, reduce_op=bass_isa.ReduceOp.add)
    xsq_r = sc.tile([128, 1], F32, tag="xsq_r")
    nc.vector.tensor_reduce(xsq_r, xsqc, axis=mybir.AxisListType.X, op=ALU.add)
    xsq_ar = sc.tile([128, 1], F32, tag="xsq_ar")
    nc.gpsimd.partition_all_reduce(xsq_ar, xsq_r, channels=128, reduce_op=bass_isa.ReduceOp.add)

    stats_sb = sc.tile([1, 2], F32, tag="stats_sb")
    nc.vector.tensor_copy(stats_sb[:, 0:1], cnt_ar[0:1, :])
    nc.vector.tensor_copy(stats_sb[:, 1:2], xsq_ar[0:1, :])
    tot = sc.tile([1, 2], F32, tag="tot")
    if int(os.environ.get("DISOUT_NOCC", "0")):
        # single-core / cost-model builds: skip the collective
        nc.vector.tensor_copy(tot, stats_sb)
        nc.vector.tensor_scalar_mul(tot, tot, float(NCORES))
    else:
        nc.gpsimd.dma_start(out=cc_in, in_=stats_sb)
        nc.gpsimd.collective_compute(
            "AllReduce",
            ALU.add,
            ins=[cc_in],
            outs=[cc_out],
            replica_groups=[list(range(NCORES))],
        )
        nc.gpsimd.dma_start(out=tot, in_=cc_out)

    r = sc.tile([1, 1], F32, tag="r")
    nc.vector.reciprocal(r, tot[:, 0:1])  # 1 / total_count
    inv_p = sc.tile([1, 1], F32, tag="inv_p")
    nc.vector.tensor_scalar_mul(inv_p, r, NF)  # 1/percent_ones
    var = sc.tile([1, 1], F32, tag="var")
    nc.vector.tensor_scalar_mul(var, tot[:, 1:2], 1.0 / NF)
    sqv = sc.tile([1, 1], F32, tag="sqv")
    nc.scalar.sqrt(sqv, var)
    tmp = sc.tile([1, 1], F32, tag="tmp")
    nc.vector.tensor_tensor(tmp, sqv, inv_p, ALU.mult)
    scale2 = sc.tile([1, 1], F32, tag="scale2")
    nc.vector.tensor_scalar_mul(scale2, tmp, 0.01)  # 0.01*sqrt(var)/p
    inv_p_b = sc.tile([128, 1], F32, tag="inv_p_b")
    nc.gpsimd.partition_broadcast(inv_p_b, inv_p)
    scale2_b = sc.tile([128, 1], F32, tag="scale2_b")
    nc.gpsimd.partition_broadcast(scale2_b, scale2)

    if DBG is not None:
        dbg_t = sc.tile([1, 8], F32, tag="dbg_t")
        nc.vector.tensor_copy(dbg_t[:, 0:1], cnt_ar[0:1, :])
        nc.vector.tensor_copy(dbg_t[:, 1:2], xsq_ar[0:1, :])
        nc.vector.tensor_copy(dbg_t[:, 2:3], tot[:, 0:1])
        nc.vector.tensor_copy(dbg_t[:, 3:4], tot[:, 1:2])
        nc.vector.tensor_copy(dbg_t[:, 4:5], inv_p)
        nc.vector.tensor_copy(dbg_t[:, 5:6], scale2)
        nc.vector.tensor_copy(dbg_t[:, 6:7], inv_p_b[96:97, :])
        nc.vector.tensor_copy(dbg_t[:, 7:8], scale2_b[96:97, :])
        nc.sync.dma_start(out=DBG, in_=dbg_t)

    # ---------------- pass 2 ----------------
    XF2 = 2 * XF  # 1568: bigger streaming chunks, half the instruction count
    NXCH2 = NXCH // 2
    bp2d = bp_dram.rearrange("b w h c -> (b w h c)").rearrange("(p f) -> p f", p=128)
    for j in range(NXCH2 if "nop2" not in abl else 0):
        cs = slice(j * XF2, (j + 1) * XF2)
        xt = p_p2.tile([128, XF2], F32, tag="x")
        nc.sync.dma_start(out=xt, in_=x2[:, cs])
        nt = p_p2.tile([128, XF2], F32, tag="n")
        nc.sync.dma_start(out=nt, in_=ns2[:, cs])
        bt = p_p2.tile([128, XF2], I8, tag="b")
        nc.sync.dma_start(out=bt, in_=bp2d[:, cs])
        # base = keep-branch x/p; overwrite dropped (mask=-1) with noise branch
        ot = p_p2.tile([128, XF2], F32, tag="o")
        nc.scalar.activation(out=ot, in_=xt, func=AF.Copy, bias=0.0, scale=inv_p_b)
        # scale the noise tile in place (saves a 5th tile -> deeper buffering)
        nc.scalar.activation(out=nt, in_=nt, func=AF.Copy, bias=0.0, scale=scale2_b)
        nc.vector.copy_predicated(out=ot, mask=bt, data=nt)
        nc.sync.dma_start(out=out2[:, cs], in_=ot)


def _build(iters=1):
    nc = bacc.Bacc(
        "TRN2",
        target_bir_lowering=False,
        debug=False,
        enable_asserts=False,
        num_devices=NCORES,
    )
    X = nc.dram_tensor("x", [BL, W, H, C], F32, kind="ExternalInput").ap()
    RD = nc.dram_tensor("randdist", [BL, W, H, C], F32, kind="ExternalInput").ap()
    NS = nc.dram_tensor("noise", [BL, W, H, C], F32, kind="ExternalInput").ap()
    OUT = nc.dram_tensor("out", [BL, W, H, C], F32, kind="ExternalOutput").ap()
    DBG = None
    if int(os.environ.get("DISOUT_DEBUG", "0")):
        DBG = nc.dram_tensor("dbg", [1, 8], F32, kind="ExternalOutput").ap()
    with tile.TileContext(nc) as tc:
        for it in range(iters):
            with ExitStack() as ctx:
                _emit(nc, tc, ctx, X, RD, NS, OUT, DBG, it=it)
    nc.compile()
    return nc


def kernel(x, randdist, noise):
    global _NC
    if _NC is None:
        _NC = _build()
    x = np.ascontiguousarray(x, dtype=np.float32)
    randdist = np.ascontiguousarray(randdist, dtype=np.float32)
    noise = np.ascontiguousarray(noise, dtype=np.float32)
    in_maps = [
        {
            "x": x[i * BL : (i + 1) * BL],
            "randdist": randdist[i * BL : (i + 1) * BL],
            "noise": noise[i * BL : (i + 1) * BL],
        }
        for i in range(NCORES)
    ]
    trace = bool(int(os.environ.get("DISOUT_TRACE", "0")))
    res = run_bass_kernel_spmd(
        _NC, in_maps, core_ids=list(range(NCORES)), trace=trace
    )
    if trace and res.exec_time_ns is not None:
        print(f"HW exec time: {res.exec_time_ns} ns")
        if res.instructions_and_trace is not None:
            print(f"trace: {res.instructions_and_trace[1]}")
    return np.concatenate([res.results[i]["out"] for i in range(NCORES)], axis=0)
